# revision 1
# baseline (speedup 1.0000x reference)
"""Radix-4 DIF ambiguity kernel.

Per batch: u_c = s[m]*conj(s[m-k]) sliding-window products (DVE, bf16),
FFT4 combine over c (DVE, bf16), then 4 branch DFT-256 matmuls with
re/im-concatenated bf16 tables (PE, 512-wide moving), |X|^2 via ACT squares
+ DVE/Pool pair adds. Normalization is exact-by-construction (Cauchy-Schwarz:
max chi = (sum|s|^2)^2) and folded into a host prescale of s. Only k in
[0,512) is computed on device; row k=512 and the mirror half-plane
chi[k,f] = chi[N-k, -f] are assembled during host-side unsharding.
"""

import numpy as np
import ml_dtypes

import bass_rust
import concourse.bass as bass
import concourse.mybir as mybir
import concourse.tile as tile
import concourse.bass_utils as bass_utils

B, N = 16, 1024
NCORES = 8
BPC = B // NCORES
K = 512
DS_LEN = 2176

f32 = mybir.dt.float32
bf16 = mybir.dt.bfloat16
ALU = mybir.AluOpType


def _split_excess_waits(nc):
    for f in nc.m.functions:
        for blk in f.blocks:
            insts = list(blk.instructions)
            new_insts = []
            changed = False
            for inst in insts:
                si = inst.sync_info
                waits = list(si.on_wait) if (si is not None and si.on_wait) else []
                keep_n = 0 if isinstance(inst, mybir.InstDrain) else 1
                if len(waits) > keep_n:
                    changed = True
                    extra = waits[: len(waits) - keep_n]
                    keep = waits[len(waits) - keep_n:]
                    for w in extra:
                        nop = mybir.InstNoOp(
                            name=nc.get_next_instruction_name(), ins=[], outs=[]
                        )
                        nop.engine = inst.engine
                        nop.sync_info = bass_rust.SyncInfo(on_wait=[w], on_update=[])
                        new_insts.append(nop)
                    inst.sync_info = bass_rust.SyncInfo(
                        on_wait=keep,
                        on_update=list(si.on_update) if si.on_update else [],
                    )
                new_insts.append(inst)
            if changed:
                blk.instructions = new_insts
    return nc


def build_nc():
    nc = bass.Bass("TRN2", target_bir_lowering=False, debug=False)

    dsr = nc.dram_tensor("dsr", [BPC, DS_LEN], bf16, kind="ExternalInput")
    dsi = nc.dram_tensor("dsi", [BPC, DS_LEN], bf16, kind="ExternalInput")
    scols = nc.dram_tensor("scols", [BPC, 128, 16], f32, kind="ExternalInput")
    tabs = {}
    for r in range(4):
        for form in "AB":
            for h in range(2):
                nm = f"t{form}{r}{h}"
                tabs[(form, r, h)] = nc.dram_tensor(nm, [128, 512], bf16, kind="ExternalInput")
    out = nc.dram_tensor("out", [BPC, K, N], f32, kind="ExternalOutput")

    with tile.TileContext(nc) as tc:
        with (
            tc.tile_pool(name="const", bufs=1) as constp,
            tc.tile_pool(name="win", bufs=2) as winp,
            tc.tile_pool(name="sm", bufs=2) as smp,
            tc.tile_pool(name="u", bufs=2) as up,
            tc.tile_pool(name="pq", bufs=3) as pqp,
            tc.tile_pool(name="bb", bufs=2) as bbp,
            tc.tile_pool(name="sq", bufs=2) as sqp,
            tc.tile_pool(name="chi", bufs=2) as chip,
            tc.tile_pool(name="ps", bufs=2, space="PSUM") as psp,
        ):
            TT = {}
            for i, key in enumerate(tabs):
                TT[key] = constp.tile([128, 512], bf16, tag=f"tab{i}", name=f"tab{i}")

            def load_tables():
                engs = [nc.sync, nc.gpsimd]
                for i, (key, dt_) in enumerate(tabs.items()):
                    engs[i % 2].dma_start(TT[key][:], dt_[:])

            def emit_load(b):
                s = {"b": b}
                Tsr = winp.tile([128, 1536], bf16, tag="tsr", name=f"tsr{b}")
                Tsi = winp.tile([128, 1536], bf16, tag="tsi", name=f"tsi{b}")
                for p0, p1 in ((0, 64), (64, 128)):
                    nc.sync.dma_start(
                        Tsr[p0:p1, :],
                        bass.AP(dsr, b * DS_LEN + 385 + p0, [[1, p1 - p0], [1, 1536]]),
                    )
                    nc.gpsimd.dma_start(
                        Tsi[p0:p1, :],
                        bass.AP(dsi, b * DS_LEN + 385 + p0, [[1, p1 - p0], [1, 1536]]),
                    )
                scol = smp.tile([128, 16], f32, tag="scol", name=f"scol{b}")
                nc.sync.dma_start(scol[:], scols[b])
                s["T"] = (Tsr, Tsi)
                s["scol"] = scol
                s["u"] = {}
                s["B"] = {}
                return s

            def win(T, j, lo, n):
                ap = T[:]
                return bass.AP(ap.tensor, ap.offset + 639 + 128 * j - lo, [ap.ap[0], [-1, n]])

            def emit_ubuild(s, js, lo, hi):
                Tsr, Tsi = s["T"]
                scol = s["scol"]
                n = hi - lo
                for j in js:
                    w_sr = win(Tsr, j, lo, n)
                    w_si = win(Tsi, j, lo, n)
                    sr_c = scol[:, j:j + 1]
                    si_c = scol[:, 8 + j:9 + j]
                    if lo == 0:
                        ut = up.tile([128, 2 * K], bf16, tag=f"u{j}", name=f"u{j}_{s['b']}")
                        s["u"][j] = ut
                    else:
                        ut = s["u"][j]
                    ure = ut[:, lo:hi]
                    uim = ut[:, K + lo:K + hi]
                    # no-stt form: ACT does the w_si muls, DVE does w_sr muls +
                    # fast-mode adds (stt has no 2x mode; tsm+tt does)
                    a = pqp.tile([128, K], bf16, tag="ta", name=f"ta{s['b']}{j}{lo}")
                    nc.scalar.mul(a[:, lo:hi], w_si, si_c)
                    b2 = pqp.tile([128, K], bf16, tag="tb", name=f"tb{s['b']}{j}{lo}")
                    nc.scalar.mul(b2[:, lo:hi], w_si, sr_c)
                    m1 = pqp.tile([128, K], bf16, tag="tm1", name=f"tm1{s['b']}{j}{lo}")
                    nc.vector.tensor_scalar_mul(m1[:, lo:hi], w_sr, sr_c)
                    nc.vector.tensor_tensor(ure, m1[:, lo:hi], a[:, lo:hi], op=ALU.add)
                    m2 = pqp.tile([128, K], bf16, tag="tm2", name=f"tm2{s['b']}{j}{lo}")
                    nc.vector.tensor_scalar_mul(m2[:, lo:hi], w_sr, si_c)
                    nc.vector.tensor_tensor(uim, m2[:, lo:hi], b2[:, lo:hi], op=ALU.subtract)

            def emit_fft4(s, h, lo, hi):
                # B_r[h-chunk] = sum_c (-i)^{cr} u_{j=2c+h}; tiles pack (re|im)
                b = s["b"]
                u0 = s["u"][h]
                u1 = s["u"][2 + h]
                u2 = s["u"][4 + h]
                u3 = s["u"][6 + h]
                t = {}
                for nm in ("P", "Q", "U", "W"):
                    t[nm] = pqp.tile([128, 2 * K], bf16, tag=f"{nm}{h}", name=f"{nm}{h}_{b}")
                Bt = {}
                for r in range(4):
                    Bt[r] = bbp.tile([128, 2 * K], bf16, tag=f"b{r}{h}", name=f"b{r}{h}_{b}")
                s["B"][h] = Bt
                tt = nc.vector.tensor_tensor
                tt(t["P"][:], u0[:], u2[:], op=ALU.add)
                tt(t["Q"][:], u0[:], u2[:], op=ALU.subtract)
                tt(t["U"][:], u1[:], u3[:], op=ALU.add)
                # W = (V.im | -V.re) where V = u1 - u3, so B1 = Q+W, B3 = Q-W
                tt(t["W"][:, 0:K], u1[:, K:2 * K], u3[:, K:2 * K], op=ALU.subtract)
                tt(t["W"][:, K:2 * K], u3[:, 0:K], u1[:, 0:K], op=ALU.subtract)
                tt(Bt[0][:], t["P"][:], t["U"][:], op=ALU.add)
                tt(Bt[2][:], t["P"][:], t["U"][:], op=ALU.subtract)
                tt(Bt[1][:], t["Q"][:], t["W"][:], op=ALU.add)
                tt(Bt[3][:], t["Q"][:], t["W"][:], op=ALU.subtract)

            def emit_kblock(s, kb):
                b = s["b"]
                c0 = 128 * kb
                chi_t = chip.tile([128, N], f32, tag=f"chi{kb % 2}", name=f"chi{b}{kb}")
                for r in range(4):
                    ps = psp.tile([128, 512], f32, tag=f"ps{r}", name=f"ps{b}{kb}{r}")
                    first = True
                    for h in range(2):
                        st = s["B"][h][r][:, c0:c0 + 128]
                        nc.tensor.matmul(ps[:], st, TT[("A", r, h)][:], start=first, stop=False)
                        first = False
                    for h in range(2):
                        st = s["B"][h][r][:, K + c0:K + c0 + 128]
                        nc.tensor.matmul(ps[:], st, TT[("B", r, h)][:], start=False, stop=(h == 1))
                    sq = sqp.tile([128, 512], f32, tag=f"sq{r}", name=f"sq{b}{kb}{r}")
                    nc.scalar.square(sq[:], ps[:])
                    cap = chi_t[:]
                    strided = bass.AP(cap.tensor, cap.offset + r, [cap.ap[0], [4, 256]])
                    eng = nc.vector if (b == 1 and kb == 3) else nc.gpsimd
                    eng.tensor_tensor(strided, sq[:, 0:256], sq[:, 256:512], op=ALU.add)
                return chi_t

            def emit_store(s, kb, chi_t):
                b = s["b"]
                eng = nc.sync if kb % 2 == 0 else nc.scalar
                eng.dma_start(out[b, 128 * kb:128 * kb + 128, :], chi_t[:])

            # ---- schedule ----
            s0 = emit_load(0)
            emit_ubuild(s0, range(8), 0, 512)
            emit_fft4(s0, 0, 0, 512)
            emit_fft4(s0, 1, 0, 512)
            s1 = emit_load(1)
            load_tables()
            c00 = emit_kblock(s0, 0)
            emit_ubuild(s1, range(3), 0, 512)
            emit_store(s0, 0, c00)
            c01 = emit_kblock(s0, 1)
            emit_ubuild(s1, range(3, 6), 0, 512)
            emit_store(s0, 1, c01)
            c02 = emit_kblock(s0, 2)
            emit_ubuild(s1, range(6, 8), 0, 512)
            emit_fft4(s1, 0, 0, 512)
            emit_store(s0, 2, c02)
            c03 = emit_kblock(s0, 3)
            emit_fft4(s1, 1, 0, 512)
            emit_store(s0, 3, c03)
            c10 = emit_kblock(s1, 0)
            emit_store(s1, 0, c10)
            c11 = emit_kblock(s1, 1)
            emit_store(s1, 1, c11)
            c12 = emit_kblock(s1, 2)
            emit_store(s1, 2, c12)
            c13 = emit_kblock(s1, 3)
            emit_store(s1, 3, c13)

    _split_excess_waits(nc)
    return nc


_NC_CACHE = {}


def _get_nc():
    if "nc" not in _NC_CACHE:
        _NC_CACHE["nc"] = build_nc()
    return _NC_CACHE["nc"]


def _get_tables():
    if "tabs" not in _NC_CACHE:
        mpp = np.arange(256, dtype=np.float64)[:, None]
        t = np.arange(256, dtype=np.float64)[None, :]
        t_sh = (t + 128) % 256
        tabs = {}
        for r in range(4):
            ang = 2.0 * np.pi * ((mpp * (r + 4 * t_sh)) % 1024) / 1024
            Mc = np.cos(ang)
            Ms = np.sin(ang)
            for h in range(2):
                sl = slice(128 * h, 128 * h + 128)
                tabs[f"tA{r}{h}"] = np.concatenate(
                    [Mc[sl], -Ms[sl]], axis=1
                ).astype(ml_dtypes.bfloat16)
                tabs[f"tB{r}{h}"] = np.concatenate(
                    [Ms[sl], Mc[sl]], axis=1
                ).astype(ml_dtypes.bfloat16)
        _NC_CACHE["tabs"] = tabs
    return _NC_CACHE["tabs"]


def _host_prep(sr, si):
    """Per-core input prep. sr/si: [BPC, N] float32 (already prescaled)."""
    dsr = np.tile(sr, (1, 3))[:, :DS_LEN].astype(ml_dtypes.bfloat16)
    dsi = np.tile(si, (1, 3))[:, :DS_LEN].astype(ml_dtypes.bfloat16)
    scols = np.concatenate(
        [
            sr.reshape(BPC, 8, 128).transpose(0, 2, 1),
            si.reshape(BPC, 8, 128).transpose(0, 2, 1),
        ],
        axis=2,
    ).astype(np.float32).copy()
    im = {"dsr": dsr, "dsi": dsi, "scols": scols}
    im.update(_get_tables())
    return im


def kernel(s_real: np.ndarray, s_imag: np.ndarray) -> np.ndarray:
    s_real = np.asarray(s_real, dtype=np.float32)
    s_imag = np.asarray(s_imag, dtype=np.float32)
    # exact normalization: max chi = (sum |s|^2)^2 (Cauchy-Schwarz, attained
    # at k=0,f=0), so prescale s by (sum|s|^2)^{-1/2}
    pw = (s_real.astype(np.float64) ** 2 + s_imag.astype(np.float64) ** 2).sum(
        axis=1, keepdims=True
    )
    g = 1.0 / np.sqrt(pw)
    sr_s = (s_real * g).astype(np.float32)
    si_s = (s_imag * g).astype(np.float32)

    nc = _get_nc()
    in_maps = [
        _host_prep(sr_s[c * BPC:(c + 1) * BPC], si_s[c * BPC:(c + 1) * BPC])
        for c in range(NCORES)
    ]
    res = bass_utils.run_bass_kernel_spmd(nc, in_maps, core_ids=list(range(NCORES)))
    chi = np.concatenate([r["out"] for r in res.results], axis=0)  # [B, 512, N]

    full = np.empty((B, N, N), dtype=np.float32)
    full[:, 512:1024, :] = chi
    # mirror: rows r in [1,512): chi[r] = flip_f(chi_direct[512 - r])
    src = chi[:, 511:0:-1, :]                      # k2 = 511..1 -> rows 1..511
    full[:, 1:512, 0] = src[:, :, 0]
    full[:, 1:512, 1:] = src[:, :, :0:-1]
    # row 0 (k=512) on host in float64
    s64 = (sr_s.astype(np.float64) + 1j * si_s.astype(np.float64))
    r512 = s64 * np.conj(np.roll(s64, 512, axis=1))
    x512 = np.fft.fft(r512, axis=1)
    full[:, 0, :] = np.fft.fftshift(
        (x512 * np.conj(x512)).real, axes=-1
    ).astype(np.float32)
    return full



# revision 4
# speedup vs baseline: 1.2176x; 1.2176x over previous
"""Radix-4 DIF ambiguity kernel, v2.

Host precomputes the lag products R[k,t] = s[t]*conj(s[t-k]) (<0.2% of total
FLOPs) and uploads them in u-tile layout; the device does the radix-4 FFT4
combine (DVE + Pool), 4-branch DFT-256 matmuls (PE, bf16, 512-wide moving
tables), |X|^2 via ACT squares + DVE pair-adds into r-plane-layout bf16
output. Normalization is exact-by-construction (Cauchy-Schwarz: max chi =
(sum|s|^2)^2) and folded into a host prescale of s. Only k in [0,512) is
computed on device; row k=512, the r-plane interleave, the f32 cast, and the
mirror half-plane chi[k,f] = chi[N-k,-f] are assembled during host-side
unsharding. The k-column axis is chunked so PE work starts ~6us in and all
engines pipeline across the two batches per core.
"""

import numpy as np
import ml_dtypes

import bass_rust
import concourse.bass as bass
import concourse.mybir as mybir
import concourse.tile as tile
import concourse.bass_utils as bass_utils

B, N = 16, 1024
NCORES = 8
BPC = B // NCORES
K = 512

f32 = mybir.dt.float32
bf16 = mybir.dt.bfloat16
ALU = mybir.AluOpType

# (batch, lo, hi) k-column chunks; small head/tail chunks for ramp/drain
CHUNKS = [
    (0, 0, 128), (0, 128, 256), (0, 256, 512),
    (1, 0, 256), (1, 256, 384), (1, 384, 512),
]

TKEYS = [(form, r, h) for r in range(4) for form in "AB" for h in range(2)]


def _split_excess_waits(nc):
    for f in nc.m.functions:
        for blk in f.blocks:
            insts = list(blk.instructions)
            new_insts = []
            changed = False
            for inst in insts:
                si = inst.sync_info
                waits = list(si.on_wait) if (si is not None and si.on_wait) else []
                keep_n = 0 if isinstance(inst, mybir.InstDrain) else 1
                if len(waits) > keep_n:
                    changed = True
                    extra = waits[: len(waits) - keep_n]
                    keep = waits[len(waits) - keep_n:]
                    for w in extra:
                        nop = mybir.InstNoOp(
                            name=nc.get_next_instruction_name(), ins=[], outs=[]
                        )
                        nop.engine = inst.engine
                        nop.sync_info = bass_rust.SyncInfo(on_wait=[w], on_update=[])
                        new_insts.append(nop)
                    inst.sync_info = bass_rust.SyncInfo(
                        on_wait=keep,
                        on_update=list(si.on_update) if si.on_update else [],
                    )
                new_insts.append(inst)
            if changed:
                blk.instructions = new_insts
    return nc


def build_nc():
    nc = bass.Bass("TRN2", target_bir_lowering=False, debug=False)

    rt = nc.dram_tensor("rt", [BPC, 1024, 1024], bf16, kind="ExternalInput")
    tabs_d = nc.dram_tensor("tabs", [128, 16 * 512], bf16, kind="ExternalInput")
    out = nc.dram_tensor("out", [BPC, K, N], bf16, kind="ExternalOutput")

    with tile.TileContext(nc) as tc:
        with (
            tc.tile_pool(name="const", bufs=1) as constp,
            tc.tile_pool(name="tmp", bufs=2) as tmpp,
            tc.tile_pool(name="bb", bufs=2) as bbp,
            tc.tile_pool(name="sq", bufs=3) as sqp,
            tc.tile_pool(name="chi", bufs=2) as chip,
            tc.tile_pool(name="ps", bufs=2, space="PSUM") as psp,
        ):
            TABS = constp.tile([128, 16 * 512], bf16, tag="tabs", name="tabs")

            def TT(form, r, h):
                i = TKEYS.index((form, r, h))
                return TABS[:, i * 512:(i + 1) * 512]

            # ---- input loads (all up front) ----
            nc.scalar.dma_start(TABS[:], tabs_d[:])
            UT = []
            for ci, (b, lo, hi) in enumerate(CHUNKS):
                C = hi - lo
                U = constp.tile([128, 16 * C], bf16, tag=f"u{ci}", name=f"u{ci}")
                # src: rt[b, 128j+p, reim*512 + lo + c]; dst: U[:, j*2C + reim*C + c]
                uap = U[:]
                for reim in range(2):
                    nc.sync.dma_start(
                        bass.AP(uap.tensor, uap.offset + reim * C,
                                [uap.ap[0], [2 * C, 8], [1, C]]),
                        bass.AP(rt, b * 1024 * 1024 + reim * 512 + lo,
                                [[1024, 128], [128 * 1024, 8], [1, C]]),
                    )
                UT.append(U)

            def emit_fft4(ci):
                b, lo, hi = CHUNKS[ci]
                C = hi - lo
                U = UT[ci]

                def u(j):
                    return U[:, j * 2 * C:(j + 1) * 2 * C]

                Bt = {}
                tt = nc.vector.tensor_tensor
                pt = nc.gpsimd.tensor_tensor
                for h in range(2):
                    u0, u1, u2, u3 = u(h), u(2 + h), u(4 + h), u(6 + h)
                    P = tmpp.tile([128, 2 * C], bf16, tag=f"P{h}", name=f"P{h}_{ci}")
                    Q = tmpp.tile([128, 2 * C], bf16, tag=f"Q{h}", name=f"Q{h}_{ci}")
                    U2 = tmpp.tile([128, 2 * C], bf16, tag=f"U{h}", name=f"U{h}_{ci}")
                    W = tmpp.tile([128, 2 * C], bf16, tag=f"W{h}", name=f"W{h}_{ci}")
                    for r in range(4):
                        Bt[(r, h)] = bbp.tile(
                            [128, 2 * C], bf16, tag=f"b{r}{h}", name=f"b{r}{h}_{ci}"
                        )
                    tt(P[:], u0, u2, op=ALU.add)
                    tt(Q[:], u0, u2, op=ALU.subtract)
                    tt(U2[:], u1, u3, op=ALU.add)
                    # W = (V.im | -V.re), V = u1 - u3; B1 = Q+W, B3 = Q-W
                    tt(W[:, 0:C], u1[:, C:2 * C], u3[:, C:2 * C], op=ALU.subtract)
                    tt(W[:, C:2 * C], u3[:, 0:C], u1[:, 0:C], op=ALU.subtract)
                    tt(Bt[(0, h)][:], P[:], U2[:], op=ALU.add)
                    tt(Bt[(1, h)][:], Q[:], W[:], op=ALU.add)
                    pt(Bt[(2, h)][:], P[:], U2[:], op=ALU.subtract)
                    pt(Bt[(3, h)][:], Q[:], W[:], op=ALU.subtract)
                return Bt

            def emit_kblock(ci, Bt, kb):
                # kb is the global kblock index (k rows 128*kb..128*kb+128)
                b, lo, hi = CHUNKS[ci]
                C = hi - lo
                c0 = 128 * kb - lo
                chi_t = chip.tile([128, N], bf16, tag="chi", name=f"chi{ci}_{kb}")
                for r in range(4):
                    ps = psp.tile([128, 512], f32, tag=f"ps{r}", name=f"ps{ci}{kb}{r}")
                    first = True
                    for h in range(2):
                        st = Bt[(r, h)][:, c0:c0 + 128]
                        nc.tensor.matmul(ps[:], st, TT("A", r, h), start=first, stop=False)
                        first = False
                    for h in range(2):
                        st = Bt[(r, h)][:, C + c0:C + c0 + 128]
                        nc.tensor.matmul(ps[:], st, TT("B", r, h), start=False, stop=(h == 1))
                    sq = sqp.tile([128, 512], bf16, tag="sq", name=f"sq{ci}{kb}{r}")
                    nc.scalar.square(sq[:], ps[:])
                    nc.vector.tensor_tensor(
                        chi_t[:, r * 256:(r + 1) * 256],
                        sq[:, 0:256], sq[:, 256:512], op=ALU.add,
                    )
                return chi_t

            def emit_store(b, kb, chi_t):
                nc.sync.dma_start(out[b, 128 * kb:128 * kb + 128, :], chi_t[:])

            # ---- schedule ----
            for ci, (b, lo, hi) in enumerate(CHUNKS):
                Bt = emit_fft4(ci)
                for kb in range(lo // 128, hi // 128):
                    chi_t = emit_kblock(ci, Bt, kb)
                    emit_store(b, kb, chi_t)

    _split_excess_waits(nc)
    return nc


_NC_CACHE = {}


def _get_nc():
    if "nc" not in _NC_CACHE:
        _NC_CACHE["nc"] = build_nc()
    return _NC_CACHE["nc"]


def _get_tables():
    if "tabs" not in _NC_CACHE:
        mpp = np.arange(256, dtype=np.float64)[:, None]
        t = np.arange(256, dtype=np.float64)[None, :]
        t_sh = (t + 128) % 256
        tabd = {}
        for r in range(4):
            ang = 2.0 * np.pi * ((mpp * (r + 4 * t_sh)) % 1024) / 1024
            Mc = np.cos(ang)
            Ms = np.sin(ang)
            for h in range(2):
                sl = slice(128 * h, 128 * h + 128)
                tabd[("A", r, h)] = np.concatenate([Mc[sl], -Ms[sl]], axis=1)
                tabd[("B", r, h)] = np.concatenate([Ms[sl], Mc[sl]], axis=1)
        big = np.concatenate([tabd[k] for k in TKEYS], axis=1)
        _NC_CACHE["tabs"] = big.astype(ml_dtypes.bfloat16)
    return _NC_CACHE["tabs"]


def _host_prep(sr, si):
    """Per-core input prep. sr/si: [BPC, N] float32 (already prescaled).

    Builds rt[b, t, reim*512 + k] = {re,im} of s[t]*conj(s[(t-k) % N])."""
    rt = np.empty((BPC, 1024, 1024), dtype=ml_dtypes.bfloat16)
    for b in range(BPC):
        s = sr[b].astype(np.complex64)
        s.imag = si[b]
        cs = np.conj(s)
        arr = np.concatenate([cs, cs])
        Wm = np.lib.stride_tricks.as_strided(
            arr[N:], shape=(N, K), strides=(arr.itemsize, -arr.itemsize))
        R = s[:, None] * Wm
        rt[b, :, 0:512] = R.real.astype(ml_dtypes.bfloat16)
        rt[b, :, 512:1024] = R.imag.astype(ml_dtypes.bfloat16)
    return {"rt": rt, "tabs": _get_tables()}


def kernel(s_real: np.ndarray, s_imag: np.ndarray) -> np.ndarray:
    s_real = np.asarray(s_real, dtype=np.float32)
    s_imag = np.asarray(s_imag, dtype=np.float32)
    # exact normalization: max chi = (sum |s|^2)^2 (Cauchy-Schwarz, attained
    # at k=0,f=0), so prescale s by (sum|s|^2)^{-1/2}
    pw = (s_real.astype(np.float64) ** 2 + s_imag.astype(np.float64) ** 2).sum(
        axis=1, keepdims=True
    )
    g = 1.0 / np.sqrt(pw)
    sr_s = (s_real * g).astype(np.float32)
    si_s = (s_imag * g).astype(np.float32)

    nc = _get_nc()
    in_maps = [
        _host_prep(sr_s[c * BPC:(c + 1) * BPC], si_s[c * BPC:(c + 1) * BPC])
        for c in range(NCORES)
    ]
    res = bass_utils.run_bass_kernel_spmd(nc, in_maps, core_ids=list(range(NCORES)))
    planes = np.concatenate([r["out"] for r in res.results], axis=0)  # [B,512,N] bf16
    # r-plane interleave: chi[k, 4q+r] = planes[k, r*256+q]
    chi = (
        planes.astype(np.float32)
        .reshape(B, K, 4, 256)
        .transpose(0, 1, 3, 2)
        .reshape(B, K, N)
    )

    full = np.empty((B, N, N), dtype=np.float32)
    full[:, 512:1024, :] = chi
    # mirror: rows r in [1,512): chi[r] = flip_f(chi_direct[512 - r])
    src = chi[:, 511:0:-1, :]
    full[:, 1:512, 0] = src[:, :, 0]
    full[:, 1:512, 1:] = src[:, :, :0:-1]
    # row 0 (k=512) on host in float64
    s64 = (sr_s.astype(np.float64) + 1j * si_s.astype(np.float64))
    r512 = s64 * np.conj(np.roll(s64, 512, axis=1))
    x512 = np.fft.fft(r512, axis=1)
    full[:, 0, :] = np.fft.fftshift(
        (x512 * np.conj(x512)).real, axes=-1
    ).astype(np.float32)
    return full


# revision 8
# speedup vs baseline: 1.2911x; 1.0604x over previous
"""Radix-4 DIF ambiguity kernel, v2.

Host precomputes the lag products R[k,t] = s[t]*conj(s[t-k]) (<0.2% of total
FLOPs) and uploads them in u-tile layout; the device does the radix-4 FFT4
combine (DVE + Pool), 4-branch DFT-256 matmuls (PE, bf16, 512-wide moving
tables), |X|^2 via ACT squares + DVE pair-adds into r-plane-layout bf16
output. Normalization is exact-by-construction (Cauchy-Schwarz: max chi =
(sum|s|^2)^2) and folded into a host prescale of s. Only k in [0,512) is
computed on device; row k=512, the r-plane interleave, the f32 cast, and the
mirror half-plane chi[k,f] = chi[N-k,-f] are assembled during host-side
unsharding. The k-column axis is chunked so PE work starts ~6us in and all
engines pipeline across the two batches per core.
"""

import numpy as np
import ml_dtypes

import bass_rust
import concourse.bass as bass
import concourse.mybir as mybir
import concourse.tile as tile
import concourse.bass_utils as bass_utils

B, N = 16, 1024
NCORES = 8
BPC = B // NCORES
K = 512

f32 = mybir.dt.float32
bf16 = mybir.dt.bfloat16
ALU = mybir.AluOpType

# (batch, lo, hi) k-column chunks; small head/tail chunks for ramp/drain
CHUNKS = [
    (0, 0, 128), (0, 128, 256), (0, 256, 512),
    (1, 0, 256), (1, 256, 384), (1, 384, 512),
]

TKEYS = [(form, r, h) for r in range(4) for form in "AB" for h in range(2)]

# per-chunk column offsets into the chunk-major rt layout
CH_OFF = []
_o = 0
for _b, _lo, _hi in CHUNKS:
    CH_OFF.append(_o)
    _o += 16 * (_hi - _lo)
assert _o == 2 * 16 * K


def _split_excess_waits(nc):
    for f in nc.m.functions:
        for blk in f.blocks:
            insts = list(blk.instructions)
            new_insts = []
            changed = False
            for inst in insts:
                si = inst.sync_info
                waits = list(si.on_wait) if (si is not None and si.on_wait) else []
                keep_n = 0 if isinstance(inst, mybir.InstDrain) else 1
                if len(waits) > keep_n:
                    changed = True
                    extra = waits[: len(waits) - keep_n]
                    keep = waits[len(waits) - keep_n:]
                    for w in extra:
                        nop = mybir.InstNoOp(
                            name=nc.get_next_instruction_name(), ins=[], outs=[]
                        )
                        nop.engine = inst.engine
                        nop.sync_info = bass_rust.SyncInfo(on_wait=[w], on_update=[])
                        new_insts.append(nop)
                    inst.sync_info = bass_rust.SyncInfo(
                        on_wait=keep,
                        on_update=list(si.on_update) if si.on_update else [],
                    )
                new_insts.append(inst)
            if changed:
                blk.instructions = new_insts
    return nc


def build_nc():
    nc = bass.Bass("TRN2", target_bir_lowering=False, debug=False)

    # chunk-major R layout: for each chunk ci, a [128, 16*C] block at CH_OFF[ci]
    # with per-partition layout [j(8), reim(2), c(C)]
    rt = nc.dram_tensor("rt", [128, 2 * 16 * K], bf16, kind="ExternalInput")
    tabs_d = nc.dram_tensor("tabs", [128, 16 * 512], bf16, kind="ExternalInput")
    out = nc.dram_tensor("out", [BPC, K, N], bf16, kind="ExternalOutput")

    with tile.TileContext(nc) as tc:
        with (
            tc.tile_pool(name="const", bufs=1) as constp,
            tc.tile_pool(name="tmp", bufs=2) as tmpp,
            tc.tile_pool(name="bb", bufs=2) as bbp,
            tc.tile_pool(name="sq", bufs=3) as sqp,
            tc.tile_pool(name="chi", bufs=2) as chip,
            tc.tile_pool(name="ps", bufs=2, space="PSUM") as psp,
        ):
            TABS = constp.tile([128, 16 * 512], bf16, tag="tabs", name="tabs")

            def TT(form, r, h):
                i = TKEYS.index((form, r, h))
                return TABS[:, i * 512:(i + 1) * 512]

            # ---- input loads (all up front) ----
            nc.scalar.dma_start(TABS[:], tabs_d[:])
            UT = []
            for ci, (b, lo, hi) in enumerate(CHUNKS):
                C = hi - lo
                U = constp.tile([128, 16 * C], bf16, tag=f"u{ci}", name=f"u{ci}")
                off = CH_OFF[ci]
                nc.sync.dma_start(
                    U[:],
                    bass.AP(rt, off, [[2 * 16 * K, 128], [1, 16 * C]]),
                )
                UT.append(U)

            def emit_fft4(ci):
                b, lo, hi = CHUNKS[ci]
                C = hi - lo
                U = UT[ci]

                def u(j):
                    return U[:, j * 2 * C:(j + 1) * 2 * C]

                Bt = {}
                tt = nc.vector.tensor_tensor
                pt = nc.gpsimd.tensor_tensor
                for h in range(2):
                    u0, u1, u2, u3 = u(h), u(2 + h), u(4 + h), u(6 + h)
                    P = tmpp.tile([128, 2 * C], bf16, tag=f"P{h}", name=f"P{h}_{ci}")
                    Q = tmpp.tile([128, 2 * C], bf16, tag=f"Q{h}", name=f"Q{h}_{ci}")
                    U2 = tmpp.tile([128, 2 * C], bf16, tag=f"U{h}", name=f"U{h}_{ci}")
                    W = tmpp.tile([128, 2 * C], bf16, tag=f"W{h}", name=f"W{h}_{ci}")
                    for r in range(4):
                        Bt[(r, h)] = bbp.tile(
                            [128, 2 * C], bf16, tag=f"b{r}{h}", name=f"b{r}{h}_{ci}"
                        )
                    tt(P[:], u0, u2, op=ALU.add)
                    tt(Q[:], u0, u2, op=ALU.subtract)
                    tt(U2[:], u1, u3, op=ALU.add)
                    # W = (V.im | -V.re), V = u1 - u3; B1 = Q+W, B3 = Q-W
                    tt(W[:, 0:C], u1[:, C:2 * C], u3[:, C:2 * C], op=ALU.subtract)
                    tt(W[:, C:2 * C], u3[:, 0:C], u1[:, 0:C], op=ALU.subtract)
                    tt(Bt[(0, h)][:], P[:], U2[:], op=ALU.add)
                    tt(Bt[(1, h)][:], Q[:], W[:], op=ALU.add)
                    pt(Bt[(2, h)][:], P[:], U2[:], op=ALU.subtract)
                    pt(Bt[(3, h)][:], Q[:], W[:], op=ALU.subtract)
                return Bt

            def emit_kblock(ci, Bt, kb):
                # kb is the global kblock index (k rows 128*kb..128*kb+128)
                b, lo, hi = CHUNKS[ci]
                C = hi - lo
                c0 = 128 * kb - lo
                chi_t = chip.tile([128, N], bf16, tag="chi", name=f"chi{ci}_{kb}")
                for r in range(4):
                    ps = psp.tile([128, 512], f32, tag=f"ps{r}", name=f"ps{ci}{kb}{r}")
                    first = True
                    for h in range(2):
                        st = Bt[(r, h)][:, c0:c0 + 128]
                        nc.tensor.matmul(ps[:], st, TT("A", r, h), start=first, stop=False)
                        first = False
                    for h in range(2):
                        st = Bt[(r, h)][:, C + c0:C + c0 + 128]
                        nc.tensor.matmul(ps[:], st, TT("B", r, h), start=False, stop=(h == 1))
                    sq = sqp.tile([128, 512], bf16, tag="sq", name=f"sq{ci}{kb}{r}")
                    nc.scalar.square(sq[:], ps[:])
                    nc.vector.tensor_tensor(
                        chi_t[:, r * 256:(r + 1) * 256],
                        sq[:, 0:256], sq[:, 256:512], op=ALU.add,
                    )
                return chi_t

            def emit_store(b, kb, chi_t):
                nc.sync.dma_start(out[b, 128 * kb:128 * kb + 128, :], chi_t[:])

            # ---- schedule ----
            for ci, (b, lo, hi) in enumerate(CHUNKS):
                Bt = emit_fft4(ci)
                for kb in range(lo // 128, hi // 128):
                    chi_t = emit_kblock(ci, Bt, kb)
                    emit_store(b, kb, chi_t)

    _split_excess_waits(nc)
    return nc


_NC_CACHE = {}


def _get_nc():
    if "nc" not in _NC_CACHE:
        _NC_CACHE["nc"] = build_nc()
    return _NC_CACHE["nc"]


def _get_tables():
    if "tabs" not in _NC_CACHE:
        mpp = np.arange(256, dtype=np.float64)[:, None]
        t = np.arange(256, dtype=np.float64)[None, :]
        t_sh = (t + 128) % 256
        tabd = {}
        for r in range(4):
            ang = 2.0 * np.pi * ((mpp * (r + 4 * t_sh)) % 1024) / 1024
            Mc = np.cos(ang)
            Ms = np.sin(ang)
            for h in range(2):
                sl = slice(128 * h, 128 * h + 128)
                tabd[("A", r, h)] = np.concatenate([Mc[sl], -Ms[sl]], axis=1)
                tabd[("B", r, h)] = np.concatenate([Ms[sl], Mc[sl]], axis=1)
        big = np.concatenate([tabd[k] for k in TKEYS], axis=1)
        _NC_CACHE["tabs"] = big.astype(ml_dtypes.bfloat16)
    return _NC_CACHE["tabs"]


def _host_prep(sr, si):
    """Per-core input prep. sr/si: [BPC, N] float32 (already prescaled).

    Chunk-major rt: for chunk ci=(b,lo,hi), block [128, 16*C] at CH_OFF[ci]
    where rt[p, off + j*2C + reim*C + c] = {re,im} R[k=lo+c, t=128j+p]."""
    Rts = []
    for b in range(BPC):
        s = sr[b].astype(np.complex64)
        s.imag = si[b]
        cs = np.conj(s)
        arr = np.concatenate([cs, cs])
        Wm = np.lib.stride_tricks.as_strided(
            arr[N:], shape=(N, K), strides=(arr.itemsize, -arr.itemsize))
        R = s[:, None] * Wm  # [t, k]
        Rb = np.empty((N, 2, K), dtype=np.float32)
        Rb[:, 0, :] = R.real
        Rb[:, 1, :] = R.imag
        Rts.append(Rb)
    rt = np.empty((128, 2 * 16 * K), dtype=ml_dtypes.bfloat16)
    for ci, (b, lo, hi) in enumerate(CHUNKS):
        C = hi - lo
        blk = Rts[b][:, :, lo:hi]                        # [t, 2, C]
        blk = blk.reshape(8, 128, 2, C).transpose(1, 0, 2, 3)  # [p, j, 2, C]
        rt[:, CH_OFF[ci]:CH_OFF[ci] + 16 * C] = blk.reshape(128, 16 * C).astype(
            ml_dtypes.bfloat16)
    return {"rt": rt, "tabs": _get_tables()}


def kernel(s_real: np.ndarray, s_imag: np.ndarray) -> np.ndarray:
    s_real = np.asarray(s_real, dtype=np.float32)
    s_imag = np.asarray(s_imag, dtype=np.float32)
    # exact normalization: max chi = (sum |s|^2)^2 (Cauchy-Schwarz, attained
    # at k=0,f=0), so prescale s by (sum|s|^2)^{-1/2}
    pw = (s_real.astype(np.float64) ** 2 + s_imag.astype(np.float64) ** 2).sum(
        axis=1, keepdims=True
    )
    g = 1.0 / np.sqrt(pw)
    sr_s = (s_real * g).astype(np.float32)
    si_s = (s_imag * g).astype(np.float32)

    nc = _get_nc()
    in_maps = [
        _host_prep(sr_s[c * BPC:(c + 1) * BPC], si_s[c * BPC:(c + 1) * BPC])
        for c in range(NCORES)
    ]
    res = bass_utils.run_bass_kernel_spmd(nc, in_maps, core_ids=list(range(NCORES)))
    planes = np.concatenate([r["out"] for r in res.results], axis=0)  # [B,512,N] bf16
    # r-plane interleave: chi[k, 4q+r] = planes[k, r*256+q]
    chi = (
        planes.astype(np.float32)
        .reshape(B, K, 4, 256)
        .transpose(0, 1, 3, 2)
        .reshape(B, K, N)
    )

    full = np.empty((B, N, N), dtype=np.float32)
    full[:, 512:1024, :] = chi
    # mirror: rows r in [1,512): chi[r] = flip_f(chi_direct[512 - r])
    src = chi[:, 511:0:-1, :]
    full[:, 1:512, 0] = src[:, :, 0]
    full[:, 1:512, 1:] = src[:, :, :0:-1]
    # row 0 (k=512) on host in float64
    s64 = (sr_s.astype(np.float64) + 1j * si_s.astype(np.float64))
    r512 = s64 * np.conj(np.roll(s64, 512, axis=1))
    x512 = np.fft.fft(r512, axis=1)
    full[:, 0, :] = np.fft.fftshift(
        (x512 * np.conj(x512)).real, axes=-1
    ).astype(np.float32)
    return full
